# revision 6
# baseline (speedup 1.0000x reference)
"""Cost-volume kernel for Trainium2 (8 NeuronCores, batch-parallel).

Problem: cost[b, o=(dy,dx), h, w] = PReLU(mean_c(c1[b,c,h,w] *
         pad(warped)[b,c,h+dy,w+dx]), alpha), 81 offsets (9x9), zero pad 4.

Strategy per core (one batch element per NeuronCore):
  - Image tiled 16x8 pixels (th x tw), M=128 pixel tile, b-major partition
    order (m = b8*16 + a).
  - TensorE computes a "gram" tile against the 24x16 warped halo:
    PSUM[m, n] = sum_c c1[c, p_m] * wpad[c, halo_n]  (K=96+96 chunks,
    N=384, bf16 inputs, fp32 accumulate).
  - The 81 cost entries of pixel (a, b8) live at n = (a+dy)*16 + (b8+dx),
    a sheared per-partition window that no SBUF AP can express, so the
    device writes the partition-uniform superset window of 144 values per
    row-group a, and the host finishes with a numpy diagonal gather +
    PReLU + 1/192 scale.

Scheduling (v8), driven by trace measurements (16 SDMA engines; packet
processing caps reads at ~16.5 B/ns/engine regardless of DMA size;
scattered-run writes run at ~9-11.5 B/ns, improving with run length):
  - The PSUM->SBUF cast copy reads the two half-tiles' gram columns
    n-major/hf-inner, interleaving the pair in staged SBUF. This doubles
    the gout contiguous run to 576B (halves SWDGE descriptor count and
    Q7 issue cost) at zero copy cost (the f32 PSUM read side has no
    2-elem/cycle contiguity bonus to lose).
  - Per-band staging (GB=1) with 3 staged buffers: gout waves lag up to
    2 bands behind compute without blocking it.
  - wpad loads in 4 chunks per k-chunk aligned to 2-band halo rows, all
    queued upfront on the sync HWDGE ring; c1 per 2-band group on the
    scalar ring, prefetched 2 groups ahead.
  - gout waves: bands 0-5 on the gpsimd SWDGE ring (writes spread over
    all 16 engines; HWDGE write packets pin to engines 0-7 where the
    read stream lives); bands 6-7 fan across all three rings to shorten
    the drain tail.
"""

import numpy as np

B, C, H, W = 8, 192, 128, 160
R = 4
TH, TW = 16, 8                    # pixel tile
HH, HWW = TH + 2 * R, TW + 2 * R  # halo 24 x 16
NCOL = HH * HWW                   # 384 matmul free dim
BANDS = H // TH                   # 8 row bands
TPB = W // TW                     # 20 tiles per band
WIN = 2 * R * HWW + TW + 2 * R    # 144 per-a superset window
PH, PW = H + 2 * R, W + 2 * R     # padded 136 x 168
K0, K1 = 96, 96                   # contraction chunks
NPAIR = TPB // 2                  # 10 tile-pairs per band
GROW = TW * NPAIR * 2 * WIN       # 23040 cols per gout row

_CACHE = {}


def _build():
    if "nc" in _CACHE:
        return _CACHE["nc"]
    import sys
    if "/opt/trn_rl_repo" not in sys.path:
        sys.path.insert(0, "/opt/trn_rl_repo")
    import concourse.mybir as mybir
    import concourse.tile as tile
    from concourse import bacc
    from concourse.bass import AP

    nc = bacc.Bacc(None, target_bir_lowering=False)
    bf16 = mybir.dt.bfloat16
    f32 = mybir.dt.float32

    # c1 pre-tiled on host: [C, band, t, m], m = b8*16 + a
    c1_d = nc.dram_tensor("c1b", [C, H * W], bf16, kind="ExternalInput")
    wp_d = nc.dram_tensor("wpad", [C, PH * PW], bf16, kind="ExternalInput")
    go_d = nc.dram_tensor("gout", [BANDS * TH, GROW], bf16,
                          kind="ExternalOutput")

    with tile.TileContext(nc) as tc:
        with (
            tc.tile_pool(name="wp", bufs=1) as wp_pool,
            tc.tile_pool(name="c1", bufs=2) as c1_pool,
            tc.tile_pool(name="st", bufs=3) as st_pool,
            tc.tile_pool(name="ps", bufs=4, space="PSUM") as ps_pool,
        ):
            # one persistent padded-warped tile per channel chunk; rows
            # [4,132) are real data, rows [0,4) and [132,136) are pad.
            wp_sb = {}
            for k, kn in enumerate((K0, K1)):
                t = wp_pool.tile([kn, PH * PW], bf16, tag=f"wp{k}")
                wp_sb[k] = t
                nc.gpsimd.memset(t[:, 0:R * PW], 0.0)
                nc.gpsimd.memset(t[:, (PH - R) * PW:PH * PW], 0.0)

            # ~1MB chunks aligned to 2-band halo boundaries: chunk ci
            # covers the halo rows of bands 2ci..2ci+1.
            WCHUNKS = ((4, 40), (40, 72), (72, 104), (104, 132))

            def load_wp_chunk(ci):
                ra, rb = WCHUNKS[ci]
                for k, (ks, kn) in enumerate(((0, K0), (K0, K1))):
                    nc.sync.dma_start(
                        wp_sb[k][:, ra * PW:rb * PW],
                        wp_d[ks:ks + kn, ra * PW:rb * PW])

            # c1 per 2-band group (~1MB per DMA, 10KB contiguous runs)
            def load_c1_group(grp):
                tiles = []
                for k, (ks, kn) in enumerate(((0, K0), (K0, K1))):
                    t = c1_pool.tile([kn, 2 * TPB * 128], bf16,
                                     tag=f"c1_{k}")
                    base = grp * 2 * TPB * 128
                    nc.scalar.dma_start(
                        t[:], c1_d[ks:ks + kn, base:base + 2 * TPB * 128])
                    tiles.append(t)
                return tiles

            for ci in range(len(WCHUNKS)):
                load_wp_chunk(ci)
            c1_tiles = {0: load_c1_group(0)}
            c1_tiles[1] = load_c1_group(1)

            # PE warm-up burst during the initial DMA window keeps the HAM
            # clock up before the real stream starts.
            warm = c1_pool.tile([128, 512], bf16, tag="warm")
            nc.gpsimd.memset(warm[:], 0.0)
            for _ in range(10):
                ps_w = ps_pool.tile([128, 1024], f32, tag="ps")
                nc.tensor.matmul(ps_w[:, 0:512], warm[:, 0:128],
                                 warm[:, 0:512], start=True, stop=True)

            for band in range(BANDS):
                grp = band // 2
                if band % 2 == 0 and grp + 2 < BANDS // 2:
                    c1_tiles[grp + 2] = load_c1_group(grp + 2)
                c1_sb = c1_tiles[grp]
                bb = band % 2
                r0 = band * TH

                staged = st_pool.tile([128, TPB * NCOL], bf16, tag="staged")
                sap0 = staged[:]
                srow = sap0.ap[0][0]

                for tp in range(NPAIR):
                    ps = ps_pool.tile([128, 1024], f32, tag="ps")
                    for hf in range(2):
                        t_i = 2 * tp + hf
                        c0 = t_i * TW
                        for k, kn in enumerate((K0, K1)):
                            a1 = c1_sb[k][:]
                            lhsT = AP(a1.tensor,
                                      a1.offset + (bb * TPB + t_i) * 128,
                                      [[a1.ap[0][0], kn], [1, 128]])
                            a2 = wp_sb[k][:]
                            rhs = AP(a2.tensor,
                                     a2.offset + r0 * PW + c0,
                                     [[a2.ap[0][0], kn],
                                      [PW, HH], [1, HWW]])
                            nc.tensor.matmul(
                                ps[:, hf * 512:hf * 512 + NCOL],
                                lhsT, rhs,
                                start=(k == 0), stop=(k == 1))
                    # one copy moves both tiles' grams, n-major/hf-inner:
                    # staged col j = 2n + hf (pair-interleaved)
                    pap = ps[:]
                    src2 = AP(pap.tensor, pap.offset,
                              [[pap.ap[0][0], 128], [1, NCOL], [512, 2]])
                    d0 = 2 * tp * NCOL
                    dst2 = staged[:, d0:d0 + 2 * NCOL]
                    if tp % 5 < 3:
                        nc.vector.tensor_copy(dst2, src2)
                    else:
                        nc.scalar.copy(dst2, src2)

                # gout wave for this band: 16 a-group DMAs, runs of
                # 2*WIN=288 elements (576B) per tile-pair
                gap = go_d[:]
                for a in range(TH):
                    src = AP(sap0.tensor,
                             sap0.offset + a * srow + 2 * a * HWW,
                             [[TH * srow, TW], [2 * NCOL, NPAIR],
                              [1, 2 * WIN]])
                    dst = AP(gap.tensor,
                             gap.offset + (band * TH + a) * GROW,
                             [[NPAIR * 2 * WIN, TW], [2 * WIN, NPAIR],
                              [1, 2 * WIN]])
                    if band < BANDS - 2:
                        eng = nc.gpsimd
                    else:
                        eng = (nc.sync, nc.scalar, nc.gpsimd)[a % 3]
                    eng.dma_start(dst, src)

    nc.finalize()
    _CACHE["nc"] = nc
    return nc


def kernel(c1, warped, alpha):
    import sys
    if "/opt/trn_rl_repo" not in sys.path:
        sys.path.insert(0, "/opt/trn_rl_repo")
    import ml_dtypes
    from concourse.bass_utils import run_bass_kernel_spmd

    nc = _build()
    bf = ml_dtypes.bfloat16

    in_maps = []
    for b in range(B):
        wpad = np.zeros((C, PH, PW), np.float32)
        wpad[:, R:R + H, R:R + W] = warped[b]
        # tile c1: [C, band, a, t, b8] -> [C, band, t, b8, a]; m = b8*16 + a
        c1t = np.asarray(c1[b]).reshape(C, BANDS, TH, TPB, TW)
        c1t = np.ascontiguousarray(c1t.transpose(0, 1, 3, 4, 2))
        in_maps.append({
            "c1b": c1t.reshape(C, H * W).astype(bf),
            "wpad": wpad.reshape(C, PH * PW).astype(bf),
        })

    import os
    trace = bool(int(os.environ.get("COSTVOL_TRACE", "0")))
    res = run_bass_kernel_spmd(nc, in_maps, core_ids=list(range(B)),
                               trace=trace)
    if trace:
        _CACHE["last_exec_time_ns"] = res.exec_time_ns

    # host-side: de-interleave + diagonal gather + mean + PReLU
    a_val = float(np.asarray(alpha).reshape(-1)[0])
    dy, dx = np.meshgrid(np.arange(9), np.arange(9), indexing="ij")
    oidx = (dy * HWW + dx).reshape(-1)                      # [81]
    # gout row (band*16+a) cols: [b8][pair][j], j = 2*(16dy+b8+dx) + hf
    jidx = (2 * (np.arange(TW)[:, None, None] + oidx[None, None, :])
            + np.arange(2)[None, :, None])                  # [b8, hf, 81]
    jflat = jidx.reshape(TW, 2 * 81)                        # [b8, 162]

    out = np.empty((B, 81, H, W), np.float32)
    for b in range(B):
        g = np.asarray(res.results[b]["gout"]).astype(np.float32)
        g = g.reshape(BANDS, TH, TW, NPAIR, 2 * WIN)
        got = np.take_along_axis(
            g, jflat[None, None, :, None, :], axis=4)
        got = got.reshape(BANDS, TH, TW, NPAIR, 2, 81)
        # -> [81, band, a, pair, hf, b8] -> [81, h, w]
        cost = got.transpose(5, 0, 1, 3, 4, 2).reshape(81, H, W) * (1.0 / C)
        out[b] = np.where(cost >= 0, cost, a_val * cost)
    return out


# revision 7
# speedup vs baseline: 1.1660x; 1.1660x over previous
"""Cost-volume kernel for Trainium2 (8 NeuronCores, batch-parallel).

Problem: cost[b, o=(dy,dx), h, w] = PReLU(mean_c(c1[b,c,h,w] *
         pad(warped)[b,c,h+dy,w+dx]), alpha), 81 offsets (9x9), zero pad 4.

Strategy per core (one batch element per NeuronCore):
  - Image tiled 16x8 pixels, M=128 pixel tile, m = b8*16 + a.
  - TensorE gram tile vs the 24x16 warped halo: PSUM[m, n] =
    sum_c c1[c, p_m] * wpad[c, halo_n] (K=96+96, N=384, bf16->fp32).
  - The 81 cost entries of pixel (a, b8) live at n = (a+dy)*16 + (b8+dx),
    a sheared per-partition window no SBUF AP can express, so the device
    writes the partition-uniform 144-superset per row-group a and the
    host finishes with a numpy diagonal gather + PReLU + 1/192 scale.

Scheduling (v9), from trace-measured DMA behavior (16 SDMA engines;
~16.5 B/ns/engine on reads regardless of DMA size; scattered-run write
rate scales with run length: 288B runs ~9-10, 576B runs ~14.9 B/ns;
SWDGE Q7 issue ~0.65us/DMA fixed, HWDGE ~0.45us; HWDGE write packets
pin to engines 0-7, SWDGE spreads over all 16):
  - PSUM->SBUF cast copy reads the tile-pair's gram columns n-major/
    hf-inner, pair-interleaving staged SBUF at zero copy cost (f32 PSUM
    reads have no contiguity bonus to lose) -> gout runs 576B.
  - Reads stay fine-grained for just-in-time pacing (12-row wpad chunks
    on sync, per-band c1 on scalar, 3 bands of prefetch): coarse 1MB
    chunks measurably bubble the matmul pipe at the same engine rate.
  - gout in 2-band waves (92KB per a-DMA amortizes the Q7 fixed cost,
    feed ~142 B/ns): groups 0-2 mostly Q7-SWDGE with 3/16 on sync;
    last group per-band across all three rings to shorten the tail.
"""

import numpy as np

B, C, H, W = 8, 192, 128, 160
R = 4
TH, TW = 16, 8                    # pixel tile
HH, HWW = TH + 2 * R, TW + 2 * R  # halo 24 x 16
NCOL = HH * HWW                   # 384 matmul free dim
BANDS = H // TH                   # 8 row bands
TPB = W // TW                     # 20 tiles per band
WIN = 2 * R * HWW + TW + 2 * R    # 144 per-a superset window
PH, PW = H + 2 * R, W + 2 * R     # padded 136 x 168
K0, K1 = 96, 96                   # contraction chunks
GB = 2                            # bands per staged group
NGRP = BANDS // GB                # 4 staged groups
NPAIR = TPB // 2                  # tile-pairs per band
GPAIR = GB * NPAIR                # 20 pairs per group
GROW = TW * GPAIR * 2 * WIN       # 46080 cols per gout row

_CACHE = {}


def _build():
    if "nc" in _CACHE:
        return _CACHE["nc"]
    import sys
    if "/opt/trn_rl_repo" not in sys.path:
        sys.path.insert(0, "/opt/trn_rl_repo")
    import concourse.mybir as mybir
    import concourse.tile as tile
    from concourse import bacc
    from concourse.bass import AP

    nc = bacc.Bacc(None, target_bir_lowering=False)
    bf16 = mybir.dt.bfloat16
    f32 = mybir.dt.float32

    # c1 pre-tiled on host: [C, band, t, m], m = b8*16 + a
    c1_d = nc.dram_tensor("c1b", [C, H * W], bf16, kind="ExternalInput")
    wp_d = nc.dram_tensor("wpad", [C, PH * PW], bf16, kind="ExternalInput")
    go_d = nc.dram_tensor("gout", [NGRP * TH, GROW], bf16,
                          kind="ExternalOutput")

    with tile.TileContext(nc) as tc:
        with (
            tc.tile_pool(name="wp", bufs=1) as wp_pool,
            tc.tile_pool(name="c1", bufs=4) as c1_pool,
            tc.tile_pool(name="st", bufs=2) as st_pool,
            tc.tile_pool(name="ps", bufs=4, space="PSUM") as ps_pool,
        ):
            # one persistent padded-warped tile per channel chunk; rows
            # [4,132) are real data, rows [0,4) and [132,136) are pad.
            wp_sb = {}
            for k, kn in enumerate((K0, K1)):
                t = wp_pool.tile([kn, PH * PW], bf16, tag=f"wp{k}")
                wp_sb[k] = t
                nc.gpsimd.memset(t[:, 0:R * PW], 0.0)
                nc.gpsimd.memset(t[:, (PH - R) * PW:PH * PW], 0.0)

            # 12-row chunks: fine-grained deps pace band compute
            WCHUNKS = tuple((r, min(r + 12, PH - R))
                            for r in range(R, PH - R, 12))

            def load_wp_chunk(ci):
                ra, rb = WCHUNKS[ci]
                for k, (ks, kn) in enumerate(((0, K0), (K0, K1))):
                    nc.sync.dma_start(
                        wp_sb[k][:, ra * PW:rb * PW],
                        wp_d[ks:ks + kn, ra * PW:rb * PW])

            def load_c1(band, eng):
                tiles = []
                for k, (ks, kn) in enumerate(((0, K0), (K0, K1))):
                    t = c1_pool.tile([kn, TPB * 128], bf16, tag=f"c1_{k}")
                    eng.dma_start(
                        t[:], c1_d[ks:ks + kn,
                                   band * TPB * 128:(band + 1) * TPB * 128])
                    tiles.append(t)
                return tiles

            # band n's halo rows [16n, 16n+24) live in chunks with
            # ra < 16n+24; pace loads two bands ahead of compute
            def wmax(n):
                return max(ci for ci, (ra, rb) in enumerate(WCHUNKS)
                           if ra < 16 * n + 24)

            load_wp_chunk(0)
            load_wp_chunk(1)
            c1_tiles = {0: load_c1(0, nc.scalar)}
            c1_tiles[1] = load_c1(1, nc.scalar)
            load_wp_chunk(2)
            c1_tiles[2] = load_c1(2, nc.scalar)
            wp_next = 3

            # PE warm-up burst during the initial DMA window keeps the HAM
            # clock up before the real stream starts.
            warm = c1_pool.tile([128, 512], bf16, tag="warm")
            nc.gpsimd.memset(warm[:], 0.0)
            for _ in range(10):
                ps_w = ps_pool.tile([128, 1024], f32, tag="ps")
                nc.tensor.matmul(ps_w[:, 0:512], warm[:, 0:128],
                                 warm[:, 0:512], start=True, stop=True)

            for grp in range(NGRP):
                staged = st_pool.tile([128, GPAIR * 2 * NCOL], bf16,
                                      tag="staged")
                sap0 = staged[:]
                srow = sap0.ap[0][0]

                for bb in range(GB):
                    band = grp * GB + bb
                    r0 = band * TH
                    c1_sb = c1_tiles.pop(band)
                    # prefetch three bands ahead; pace wp chunks likewise
                    if band + 3 < BANDS:
                        c1_tiles[band + 3] = load_c1(band + 3, nc.scalar)
                    while wp_next <= wmax(min(band + 2, BANDS - 1)):
                        load_wp_chunk(wp_next)
                        wp_next += 1

                    for tp in range(NPAIR):
                        ps = ps_pool.tile([128, 1024], f32, tag="ps")
                        for hf in range(2):
                            t_i = 2 * tp + hf
                            c0 = t_i * TW
                            for k, kn in enumerate((K0, K1)):
                                a1 = c1_sb[k][:]
                                lhsT = AP(a1.tensor,
                                          a1.offset + t_i * 128,
                                          [[a1.ap[0][0], kn], [1, 128]])
                                a2 = wp_sb[k][:]
                                rhs = AP(a2.tensor,
                                         a2.offset + r0 * PW + c0,
                                         [[a2.ap[0][0], kn],
                                          [PW, HH], [1, HWW]])
                                nc.tensor.matmul(
                                    ps[:, hf * 512:hf * 512 + NCOL],
                                    lhsT, rhs,
                                    start=(k == 0), stop=(k == 1))
                        # one copy moves both tiles' grams n-major/
                        # hf-inner: staged col j = 2n + hf (interleaved)
                        pap = ps[:]
                        src2 = AP(pap.tensor, pap.offset,
                                  [[pap.ap[0][0], 128], [1, NCOL],
                                   [512, 2]])
                        d0 = (bb * NPAIR + tp) * 2 * NCOL
                        dst2 = staged[:, d0:d0 + 2 * NCOL]
                        if tp % 5 < 3:
                            nc.vector.tensor_copy(dst2, src2)
                        else:
                            nc.scalar.copy(dst2, src2)

                # gout waves: 16 a-group DMAs per 2-band group, runs of
                # 2*WIN=288 elements (576B) per tile-pair
                gap = go_d[:]
                if grp == NGRP - 1:
                    # reads are done by now: per-band across all three
                    # rings for parallel descriptor feed + short drain
                    rings = (nc.sync, nc.scalar, nc.gpsimd)
                    for bb in range(GB):
                        for a in range(TH):
                            src = AP(sap0.tensor,
                                     sap0.offset + a * srow + 2 * a * HWW
                                     + bb * NPAIR * 2 * NCOL,
                                     [[TH * srow, TW], [2 * NCOL, NPAIR],
                                      [1, 2 * WIN]])
                            dst = AP(gap.tensor,
                                     gap.offset + (grp * TH + a) * GROW
                                     + bb * NPAIR * 2 * WIN,
                                     [[GPAIR * 2 * WIN, TW],
                                      [2 * WIN, NPAIR], [1, 2 * WIN]])
                            rings[a % 3].dma_start(dst, src)
                else:
                    # 13 of 16 on the Q7-SWDGE ring (92KB per DMA
                    # amortizes the ~0.65us issue; writes spread over all
                    # 16 engines), 3 on sync
                    for a in range(TH):
                        src = AP(sap0.tensor,
                                 sap0.offset + a * srow + 2 * a * HWW,
                                 [[TH * srow, TW], [2 * NCOL, GPAIR],
                                  [1, 2 * WIN]])
                        dst = AP(gap.tensor,
                                 gap.offset + (grp * TH + a) * GROW,
                                 [[GPAIR * 2 * WIN, TW], [2 * WIN, GPAIR],
                                  [1, 2 * WIN]])
                        eng = nc.sync if a % 5 == 2 else nc.gpsimd
                        eng.dma_start(dst, src)

    nc.finalize()
    _CACHE["nc"] = nc
    return nc


def kernel(c1, warped, alpha):
    import sys
    if "/opt/trn_rl_repo" not in sys.path:
        sys.path.insert(0, "/opt/trn_rl_repo")
    import ml_dtypes
    from concourse.bass_utils import run_bass_kernel_spmd

    nc = _build()
    bf = ml_dtypes.bfloat16

    in_maps = []
    for b in range(B):
        wpad = np.zeros((C, PH, PW), np.float32)
        wpad[:, R:R + H, R:R + W] = warped[b]
        # tile c1: [C, band, a, t, b8] -> [C, band, t, b8, a]; m = b8*16 + a
        c1t = np.asarray(c1[b]).reshape(C, BANDS, TH, TPB, TW)
        c1t = np.ascontiguousarray(c1t.transpose(0, 1, 3, 4, 2))
        in_maps.append({
            "c1b": c1t.reshape(C, H * W).astype(bf),
            "wpad": wpad.reshape(C, PH * PW).astype(bf),
        })

    import os
    trace = bool(int(os.environ.get("COSTVOL_TRACE", "0")))
    res = run_bass_kernel_spmd(nc, in_maps, core_ids=list(range(B)),
                               trace=trace)
    if trace:
        _CACHE["last_exec_time_ns"] = res.exec_time_ns

    # host-side: de-interleave + diagonal gather + mean + PReLU
    a_val = float(np.asarray(alpha).reshape(-1)[0])
    dy, dx = np.meshgrid(np.arange(9), np.arange(9), indexing="ij")
    oidx = (dy * HWW + dx).reshape(-1)                      # [81]
    # gout row (grp*16+a) cols: [b8][pair(bb,tp)][j], j=2*(16dy+b8+dx)+hf
    jidx = (2 * (np.arange(TW)[:, None, None] + oidx[None, None, :])
            + np.arange(2)[None, :, None])                  # [b8, hf, 81]
    jflat = jidx.reshape(TW, 2 * 81)                        # [b8, 162]

    out = np.empty((B, 81, H, W), np.float32)
    for b in range(B):
        g = np.asarray(res.results[b]["gout"]).astype(np.float32)
        g = g.reshape(NGRP, TH, TW, GB, NPAIR, 2 * WIN)
        got = np.take_along_axis(
            g, jflat[None, None, :, None, None, :], axis=5)
        got = got.reshape(NGRP, TH, TW, GB, NPAIR, 2, 81)
        # axes [grp, a, b8, bb, tp, hf, o] -> [o, grp, bb, a, tp, hf, b8]
        cost = got.transpose(6, 0, 3, 1, 4, 5, 2).reshape(81, H, W) \
            * (1.0 / C)
        out[b] = np.where(cost >= 0, cost, a_val * cost)
    return out


# revision 11
# speedup vs baseline: 1.2032x; 1.0319x over previous
"""Cost-volume kernel for Trainium2 (8 NeuronCores, batch-parallel).

Problem: cost[b, o=(dy,dx), h, w] = PReLU(mean_c(c1[b,c,h,w] *
         pad(warped)[b,c,h+dy,w+dx]), alpha), 81 offsets (9x9), zero pad 4.

Strategy per core (one batch element per NeuronCore):
  - Image tiled 16x8 pixels, M=128 pixel tile, m = b8*16 + a.
  - TensorE gram tile vs the 24x16 warped halo: PSUM[m, n] =
    sum_c c1[c, p_m] * wpad[c, halo_n] (K=96+96, N=384, bf16->fp32).
  - The 81 cost entries of pixel (a, b8) live at n = (a+dy)*16 + (b8+dx),
    a sheared per-partition window no SBUF AP can express, so the device
    writes the partition-uniform 144-superset per row-group a and the
    host finishes with a numpy diagonal gather + PReLU + 1/192 scale.

Scheduling (v9), from trace-measured DMA behavior (16 SDMA engines;
~16.5 B/ns/engine on reads regardless of DMA size; scattered-run write
rate scales with run length: 288B runs ~9-10, 576B runs ~14.9 B/ns;
SWDGE Q7 issue ~0.65us/DMA fixed, HWDGE ~0.45us; HWDGE write packets
pin to engines 0-7, SWDGE spreads over all 16):
  - PSUM->SBUF cast copy reads the tile-pair's gram columns n-major/
    hf-inner, pair-interleaving staged SBUF at zero copy cost (f32 PSUM
    reads have no contiguity bonus to lose) -> gout runs 576B.
  - Reads stay fine-grained for just-in-time pacing (12-row wpad chunks
    on sync, per-band c1 on scalar, 3 bands of prefetch): coarse 1MB
    chunks measurably bubble the matmul pipe at the same engine rate.
  - gout in 2-band waves (92KB per a-DMA amortizes the Q7 fixed cost,
    feed ~142 B/ns): groups 0-2 mostly Q7-SWDGE with 3/16 on sync;
    last group per-band across all three rings to shorten the tail.
"""

import numpy as np

B, C, H, W = 8, 192, 128, 160
R = 4
TH, TW = 16, 8                    # pixel tile
HH, HWW = TH + 2 * R, TW + 2 * R  # halo 24 x 16
NCOL = HH * HWW                   # 384 matmul free dim
BANDS = H // TH                   # 8 row bands
TPB = W // TW                     # 20 tiles per band
WIN = 2 * R * HWW + TW + 2 * R    # 144 per-a superset window
PH, PW = H + 2 * R, W + 2 * R     # padded 136 x 168
K0, K1 = 96, 96                   # contraction chunks
GB = 2                            # bands per staged group
NGRP = BANDS // GB                # 4 staged groups
NPAIR = TPB // 2                  # tile-pairs per band
GPAIR = GB * NPAIR                # 20 pairs per group
GROW = TW * GPAIR * 2 * WIN       # 46080 cols per gout row

_CACHE = {}


def _build():
    if "nc" in _CACHE:
        return _CACHE["nc"]
    import sys
    if "/opt/trn_rl_repo" not in sys.path:
        sys.path.insert(0, "/opt/trn_rl_repo")
    import concourse.mybir as mybir
    import concourse.tile as tile
    from concourse import bacc
    from concourse.bass import AP

    nc = bacc.Bacc(None, target_bir_lowering=False)
    bf16 = mybir.dt.bfloat16
    f32 = mybir.dt.float32

    # c1 pre-tiled on host: [C, band, t, m], m = b8*16 + a
    c1_d = nc.dram_tensor("c1b", [C, H * W], bf16, kind="ExternalInput")
    wp_d = nc.dram_tensor("wpad", [C, PH * PW], bf16, kind="ExternalInput")
    go_d = nc.dram_tensor("gout", [NGRP * TH, GROW], bf16,
                          kind="ExternalOutput")

    with tile.TileContext(nc) as tc:
        with (
            tc.tile_pool(name="wp", bufs=1) as wp_pool,
            tc.tile_pool(name="c1", bufs=4) as c1_pool,
            tc.tile_pool(name="st", bufs=1) as st_pool,
            tc.tile_pool(name="ps", bufs=4, space="PSUM") as ps_pool,
        ):
            # one persistent padded-warped tile per channel chunk; rows
            # [4,132) are real data, rows [0,4) and [132,136) are pad.
            wp_sb = {}
            for k, kn in enumerate((K0, K1)):
                t = wp_pool.tile([kn, PH * PW], bf16, tag=f"wp{k}")
                wp_sb[k] = t
                nc.gpsimd.memset(t[:, 0:R * PW], 0.0)
                nc.gpsimd.memset(t[:, (PH - R) * PW:PH * PW], 0.0)

            # 12-row chunks: fine-grained deps pace band compute
            WCHUNKS = tuple((r, min(r + 12, PH - R))
                            for r in range(R, PH - R, 12))

            def load_wp_chunk(ci):
                ra, rb = WCHUNKS[ci]
                for k, (ks, kn) in enumerate(((0, K0), (K0, K1))):
                    nc.sync.dma_start(
                        wp_sb[k][:, ra * PW:rb * PW],
                        wp_d[ks:ks + kn, ra * PW:rb * PW])

            def load_c1(band, eng):
                tiles = []
                for k, (ks, kn) in enumerate(((0, K0), (K0, K1))):
                    t = c1_pool.tile([kn, TPB * 128], bf16, tag=f"c1_{k}")
                    eng.dma_start(
                        t[:], c1_d[ks:ks + kn,
                                   band * TPB * 128:(band + 1) * TPB * 128])
                    tiles.append(t)
                return tiles

            # band n's halo rows [16n, 16n+24) live in chunks with
            # ra < 16n+24; pace loads two bands ahead of compute
            def wmax(n):
                return max(ci for ci, (ra, rb) in enumerate(WCHUNKS)
                           if ra < 16 * n + 24)

            load_wp_chunk(0)
            load_wp_chunk(1)
            c1_tiles = {0: load_c1(0, nc.scalar)}
            c1_tiles[1] = load_c1(1, nc.scalar)
            load_wp_chunk(2)
            c1_tiles[2] = load_c1(2, nc.scalar)
            wp_next = 3

            # PE warm-up burst during the initial DMA window keeps the HAM
            # clock up before the real stream starts.
            warm = c1_pool.tile([128, 512], bf16, tag="warm")
            nc.gpsimd.memset(warm[:], 0.0)
            for _ in range(10):
                ps_w = ps_pool.tile([128, 1024], f32, tag="ps")
                nc.tensor.matmul(ps_w[:, 0:512], warm[:, 0:128],
                                 warm[:, 0:512], start=True, stop=True)

            # one persistent staged tile = ring of 4 per-band slots;
            # range-granular deps mean band b's copies wait only on band
            # b-4's gout reads, not a whole pool buffer (no group stall)
            SLOT = NPAIR * 2 * NCOL
            staged = st_pool.tile([128, 4 * SLOT], bf16, tag="staged")
            sap0 = staged[:]
            srow = sap0.ap[0][0]

            for grp in range(NGRP):
                gbase = (grp % 2) * 2 * SLOT  # slots {0,1} or {2,3}

                for bb in range(GB):
                    band = grp * GB + bb
                    r0 = band * TH
                    c1_sb = c1_tiles.pop(band)
                    # prefetch three bands ahead; pace wp chunks likewise
                    if band + 3 < BANDS:
                        c1_tiles[band + 3] = load_c1(band + 3, nc.scalar)
                    while wp_next <= wmax(min(band + 2, BANDS - 1)):
                        load_wp_chunk(wp_next)
                        wp_next += 1

                    for tp in range(NPAIR):
                        ps = ps_pool.tile([128, 1024], f32, tag="ps")
                        for hf in range(2):
                            t_i = 2 * tp + hf
                            c0 = t_i * TW
                            for k, kn in enumerate((K0, K1)):
                                a1 = c1_sb[k][:]
                                lhsT = AP(a1.tensor,
                                          a1.offset + t_i * 128,
                                          [[a1.ap[0][0], kn], [1, 128]])
                                a2 = wp_sb[k][:]
                                rhs = AP(a2.tensor,
                                         a2.offset + r0 * PW + c0,
                                         [[a2.ap[0][0], kn],
                                          [PW, HH], [1, HWW]])
                                nc.tensor.matmul(
                                    ps[:, hf * 512:hf * 512 + NCOL],
                                    lhsT, rhs,
                                    start=(k == 0), stop=(k == 1))
                        # one copy moves both tiles' grams n-major/
                        # hf-inner: staged col j = 2n + hf (interleaved)
                        pap = ps[:]
                        src2 = AP(pap.tensor, pap.offset,
                                  [[pap.ap[0][0], 128], [1, NCOL],
                                   [512, 2]])
                        d0 = gbase + (bb * NPAIR + tp) * 2 * NCOL
                        dst2 = staged[:, d0:d0 + 2 * NCOL]
                        if tp % 5 < 3:
                            nc.vector.tensor_copy(dst2, src2)
                        else:
                            nc.scalar.copy(dst2, src2)

                # gout waves: 16 a-group DMAs per 2-band group, runs of
                # 2*WIN=288 elements (576B) per tile-pair
                gap = go_d[:]
                if grp == NGRP - 1:
                    # reads are done by now: per-band across all three
                    # rings for parallel descriptor feed + short drain
                    rings = (nc.sync, nc.scalar, nc.gpsimd)
                    for bb in range(GB):
                        for a in range(TH):
                            src = AP(sap0.tensor,
                                     sap0.offset + gbase + a * srow
                                     + 2 * a * HWW
                                     + bb * NPAIR * 2 * NCOL,
                                     [[TH * srow, TW], [2 * NCOL, NPAIR],
                                      [1, 2 * WIN]])
                            dst = AP(gap.tensor,
                                     gap.offset + (grp * TH + a) * GROW
                                     + bb * NPAIR * 2 * WIN,
                                     [[GPAIR * 2 * WIN, TW],
                                      [2 * WIN, NPAIR], [1, 2 * WIN]])
                            rings[a % 3].dma_start(dst, src)
                else:
                    # 2-band waves, 92KB per a-DMA (amortizes the ~0.65us
                    # Q7 issue; SWDGE writes spread over all 16 engines).
                    # Early groups lean on Q7 with 3/16 on sync; group 2
                    # adds a scalar share as the read streams taper.
                    for a in range(TH):
                        src = AP(sap0.tensor,
                                 sap0.offset + gbase + a * srow
                                 + 2 * a * HWW,
                                 [[TH * srow, TW], [2 * NCOL, GPAIR],
                                  [1, 2 * WIN]])
                        dst = AP(gap.tensor,
                                 gap.offset + (grp * TH + a) * GROW,
                                 [[GPAIR * 2 * WIN, TW], [2 * WIN, GPAIR],
                                  [1, 2 * WIN]])
                        if grp < 2:
                            eng = nc.sync if a % 5 == 2 else nc.gpsimd
                        else:
                            eng = (nc.gpsimd, nc.gpsimd, nc.sync,
                                   nc.gpsimd, nc.scalar)[a % 5]
                        eng.dma_start(dst, src)

    nc.finalize()
    _CACHE["nc"] = nc
    return nc


def kernel(c1, warped, alpha):
    import sys
    if "/opt/trn_rl_repo" not in sys.path:
        sys.path.insert(0, "/opt/trn_rl_repo")
    import ml_dtypes
    from concourse.bass_utils import run_bass_kernel_spmd

    nc = _build()
    bf = ml_dtypes.bfloat16

    in_maps = []
    for b in range(B):
        wpad = np.zeros((C, PH, PW), np.float32)
        wpad[:, R:R + H, R:R + W] = warped[b]
        # tile c1: [C, band, a, t, b8] -> [C, band, t, b8, a]; m = b8*16 + a
        c1t = np.asarray(c1[b]).reshape(C, BANDS, TH, TPB, TW)
        c1t = np.ascontiguousarray(c1t.transpose(0, 1, 3, 4, 2))
        in_maps.append({
            "c1b": c1t.reshape(C, H * W).astype(bf),
            "wpad": wpad.reshape(C, PH * PW).astype(bf),
        })

    import os
    trace = bool(int(os.environ.get("COSTVOL_TRACE", "0")))
    res = run_bass_kernel_spmd(nc, in_maps, core_ids=list(range(B)),
                               trace=trace)
    if trace:
        _CACHE["last_exec_time_ns"] = res.exec_time_ns

    # host-side: de-interleave + diagonal gather + mean + PReLU
    a_val = float(np.asarray(alpha).reshape(-1)[0])
    dy, dx = np.meshgrid(np.arange(9), np.arange(9), indexing="ij")
    oidx = (dy * HWW + dx).reshape(-1)                      # [81]
    # gout row (grp*16+a) cols: [b8][pair(bb,tp)][j], j=2*(16dy+b8+dx)+hf
    jidx = (2 * (np.arange(TW)[:, None, None] + oidx[None, None, :])
            + np.arange(2)[None, :, None])                  # [b8, hf, 81]
    jflat = jidx.reshape(TW, 2 * 81)                        # [b8, 162]

    out = np.empty((B, 81, H, W), np.float32)
    for b in range(B):
        g = np.asarray(res.results[b]["gout"]).astype(np.float32)
        g = g.reshape(NGRP, TH, TW, GB, NPAIR, 2 * WIN)
        got = np.take_along_axis(
            g, jflat[None, None, :, None, None, :], axis=5)
        got = got.reshape(NGRP, TH, TW, GB, NPAIR, 2, 81)
        # axes [grp, a, b8, bb, tp, hf, o] -> [o, grp, bb, a, tp, hf, b8]
        cost = got.transpose(6, 0, 3, 1, 4, 5, 2).reshape(81, H, W) \
            * (1.0 / C)
        out[b] = np.where(cost >= 0, cost, a_val * cost)
    return out
